# revision 2
# baseline (speedup 1.0000x reference)
"""ConvLSTM2D (Keras gate order, hard_sigmoid) + inference BatchNorm on 8
Trainium2 NeuronCores.

Sharding: batch (2) x H-slabs (4) -> 8 cores, fully local. The sequential
T=16 recurrence needs neighbor rows of h each step; instead of exchanging
halos we compute a shrinking halo: at step t each core computes rows
[r0-(16-t), r1+(16-t)) so the final 16 own rows are exact. Rows outside the
global image are computed-but-masked-to-zero so one uniform SPMD program
serves all cores (edge behavior is data: zero-padded x + per-core mask).

Layout: channels-on-partitions. zin (128 x 3300 bf16) holds x_t on
partitions 0-63 and h_{t-1} on 64-127, rows width-padded to 66 cols with
zero guard cols; a 3x3 conv tap (dy,dx) is the single col offset dy*66+dx.
One matmul contracts x AND h channels at once (lhsT = [Wx_tap; Wh_tap]),
so z_t = conv(x,Wx)+conv(h,Wh) is 9 taps x 2 gate-halves = 18 accumulating
matmuls per pixel tile into PSUM (full 128x128 PE utilization).
"""
import math
import numpy as np
import ml_dtypes

import concourse.bass as bass
import concourse.mybir as mybir
import concourse.tile as tile
from concourse.bass_utils import run_bass_kernel_spmd

BF16 = ml_dtypes.bfloat16
F32 = np.float32

T, F, C, W = 16, 64, 64, 64
NR = 50            # buffer rows: [r0-17, r1+17)
WP = W + 2         # width-padded row (guard col each side)
NCOL = NR * WP     # 3300
OWN_LO, OWN_HI = 17 * WP, 33 * WP   # own 16 rows within the buffer
TAPS = [(dy, dx) for dy in (-1, 0, 1) for dx in (-1, 0, 1)]

TRACE_SIM = False
_PROG = None
_LAST_TC = None

# ---------------------------------------------------------------------------
# Workaround: this walrus build accepts at most ONE sync wait per
# instruction; Tile attaches several. Hoist extras onto same-engine NOPs
# inserted right before the instruction (per-engine order preserved).
_MAX_WAITS = 1


def _split_multi_waits(nc):
    for fn in nc.m.functions:
        for bb in fn.blocks:
            lst = bb.instructions
            out, changed = [], False
            for ins in lst:
                si = ins.sync_info
                if si is not None and len(si.on_wait) > _MAX_WAITS:
                    waits = list(si.on_wait)
                    extra, keep = waits[:-_MAX_WAITS], waits[-_MAX_WAITS:]
                    for j, w in enumerate(extra):
                        nop = mybir.InstNoOp(
                            name=f"{ins.name}.sw{j}", ins=[], outs=[],
                            text_hint="split_wait", bass_nofuse=True)
                        nop.engine = ins.engine
                        nop.sync_info = mybir.SyncInfo(on_wait=[w], on_update=[])
                        out.append(nop)
                    ins.sync_info = mybir.SyncInfo(
                        on_wait=keep, on_update=list(si.on_update))
                    changed = True
                out.append(ins)
            if changed:
                try:
                    bb.instructions = out
                except Exception:
                    lst.clear()
                    lst.extend(out)


def _chunks(c0, c1, maxn=512):
    L = c1 - c0
    n = max(1, math.ceil(L / maxn))
    base, rem = divmod(L, n)
    sizes = [base + (1 if i < rem else 0) for i in range(n)]
    out, p = [], c0
    for s in sizes:
        out.append((p, s))
        p += s
    return out


def _build():
    nc = bass.Bass(target_bir_lowering=False)
    f32, bf16 = mybir.dt.float32, mybir.dt.bfloat16

    xT_d = nc.dram_tensor("xT", [T, C, NCOL], bf16, kind="ExternalInput")
    w_d = nc.dram_tensor("w", [128, 18 * 128], bf16, kind="ExternalInput")
    mask_d = nc.dram_tensor("mask", [F, NCOL], f32, kind="ExternalInput")
    hsbif_d = nc.dram_tensor("hsb_if", [128, 1], f32, kind="ExternalInput")
    bg_d = nc.dram_tensor("bg", [F, 1], f32, kind="ExternalInput")
    hsbo_d = nc.dram_tensor("hsb_o", [F, 1], f32, kind="ExternalInput")
    bns_d = nc.dram_tensor("bns", [F, 1], f32, kind="ExternalInput")
    bnb_d = nc.dram_tensor("bnb", [F, 1], f32, kind="ExternalInput")
    out_d = nc.dram_tensor("out", [T, F, 16 * WP], f32, kind="ExternalOutput")

    Relu = mybir.ActivationFunctionType.Relu
    Tanh = mybir.ActivationFunctionType.Tanh
    Ident = mybir.ActivationFunctionType.Identity

    with tile.TileContext(nc, trace_sim=TRACE_SIM) as tc:
        with (
            tc.tile_pool(name="const", bufs=1) as cpool,
            tc.tile_pool(name="state", bufs=1) as spool,
            tc.tile_pool(name="work", bufs=3) as wpool,
            tc.tile_pool(name="ostage", bufs=2) as opool,
            tc.psum_pool(name="ps", bufs=2) as pspool,
        ):
            w_sb = cpool.tile([128, 18 * 128], bf16)
            mask_sb = cpool.tile([F, NCOL], f32)
            hsbif_sb = cpool.tile([128, 1], f32)
            bg_sb = cpool.tile([F, 1], f32)
            hsbo_sb = cpool.tile([F, 1], f32)
            bns_sb = cpool.tile([F, 1], f32)
            bnb_sb = cpool.tile([F, 1], f32)
            nc.sync.dma_start(w_sb[:], w_d[:])
            nc.sync.dma_start(mask_sb[:], mask_d[:])
            nc.sync.dma_start(hsbif_sb[:], hsbif_d[:])
            nc.sync.dma_start(bg_sb[:], bg_d[:])
            nc.sync.dma_start(hsbo_sb[:], hsbo_d[:])
            nc.sync.dma_start(bns_sb[:], bns_d[:])
            nc.sync.dma_start(bnb_sb[:], bnb_d[:])

            zin = [spool.tile([128, NCOL], bf16, name=f"zin{i}", tag=f"zin{i}")
                   for i in range(2)]
            c_sb = spool.tile([F, NCOL], f32, tag="cstate")
            nc.vector.memset(zin[0][64:128, :], 0.0)
            nc.vector.memset(zin[1][64:128, :], 0.0)
            nc.gpsimd.memset(c_sb[:], 0.0)

            for t in range(1, T + 1):
                cur = zin[(t - 1) % 2]
                nxt = zin[t % 2]
                # x_t into the current buffer's top half (rows [t, 50-t))
                xc0, xc1 = t * WP, (NR - t) * WP
                nc.sync.dma_start(cur[0:64, xc0:xc1], xT_d[t - 1, :, xc0:xc1])

                stage = opool.tile([F, 16 * WP], f32, tag="ostage")
                for p0, n in _chunks((t + 1) * WP, (NR - 1 - t) * WP):
                    ps_if = pspool.tile([128, n], f32, tag="psif")
                    ps_go = pspool.tile([128, n], f32, tag="psgo")
                    for k, (dy, dx) in enumerate(TAPS):
                        off = p0 + dy * WP + dx
                        nc.tensor.matmul(
                            ps_if[:], w_sb[:, k * 128:(k + 1) * 128],
                            cur[:, off:off + n], start=(k == 0), stop=(k == 8))
                    for k, (dy, dx) in enumerate(TAPS):
                        off = p0 + dy * WP + dx
                        nc.tensor.matmul(
                            ps_go[:], w_sb[:, 1152 + k * 128:1152 + (k + 1) * 128],
                            cur[:, off:off + n], start=(k == 0), stop=(k == 8))

                    sig_i = wpool.tile([F, n], f32, tag="sig_i")
                    sig_f = wpool.tile([F, n], f32, tag="sig_f")
                    tanh_g = wpool.tile([F, n], f32, tag="tanh_g")
                    sig_o = wpool.tile([F, n], f32, tag="sig_o")
                    t1 = wpool.tile([F, n], f32, tag="t1")
                    t2 = wpool.tile([F, n], f32, tag="t2")
                    tanh_c = wpool.tile([F, n], f32, tag="tanh_c")
                    h32 = wpool.tile([F, n], f32, tag="h32")

                    nc.scalar.activation(sig_i[:], ps_if[0:64, :], Relu,
                                         bias=hsbif_sb[0:64, 0:1], scale=0.2)
                    nc.scalar.activation(sig_f[:], ps_if[64:128, :], Relu,
                                         bias=hsbif_sb[64:128, 0:1], scale=0.2)
                    nc.vector.tensor_scalar_min(sig_i[:], sig_i[:], 1.0)
                    nc.vector.tensor_scalar_min(sig_f[:], sig_f[:], 1.0)
                    nc.scalar.activation(tanh_g[:], ps_go[0:64, :], Tanh,
                                         bias=bg_sb[:, 0:1], scale=1.0)
                    nc.scalar.activation(sig_o[:], ps_go[64:128, :], Relu,
                                         bias=hsbo_sb[:, 0:1], scale=0.2)
                    nc.vector.tensor_scalar_min(sig_o[:], sig_o[:], 1.0)
                    nc.vector.tensor_mul(t1[:], sig_i[:], tanh_g[:])
                    nc.vector.tensor_mul(t2[:], sig_f[:], c_sb[:, p0:p0 + n])
                    nc.vector.tensor_add(c_sb[:, p0:p0 + n], t1[:], t2[:])
                    nc.scalar.activation(tanh_c[:], c_sb[:, p0:p0 + n], Tanh)
                    nc.vector.tensor_mul(h32[:], sig_o[:], tanh_c[:])
                    if t < T:
                        nc.vector.tensor_mul(nxt[64:128, p0:p0 + n],
                                             h32[:], mask_sb[:, p0:p0 + n])
                    lo, hi = max(p0, OWN_LO), min(p0 + n, OWN_HI)
                    if lo < hi:
                        nc.scalar.activation(
                            stage[:, lo - OWN_LO:hi - OWN_LO],
                            h32[:, lo - p0:hi - p0], Ident,
                            bias=bnb_sb[:, 0:1], scale=bns_sb[:, 0:1])
                nc.sync.dma_start(out_d[t - 1], stage[:])

        global _LAST_TC
        _LAST_TC = tc
    _split_multi_waits(nc)
    return nc


def _prep_inputs(x, Wx, Wh, b, gamma, beta, moving_mean, moving_var):
    x = np.asarray(x, F32)
    Wx = np.asarray(Wx, F32)
    Wh = np.asarray(Wh, F32)
    b = np.asarray(b, F32)
    wstack = np.zeros((128, 18 * 128), F32)
    for k, (dy, dx) in enumerate(TAPS):
        ky, kx = dy + 1, dx + 1
        wstack[0:64, k * 128:(k + 1) * 128] = Wx[ky, kx, :, 0:128]
        wstack[64:128, k * 128:(k + 1) * 128] = Wh[ky, kx, :, 0:128]
        wstack[0:64, 1152 + k * 128:1152 + (k + 1) * 128] = Wx[ky, kx, :, 128:256]
        wstack[64:128, 1152 + k * 128:1152 + (k + 1) * 128] = Wh[ky, kx, :, 128:256]
    wstack = wstack.astype(BF16)

    hsb_if = (0.2 * b[0:128] + 0.5).reshape(128, 1).astype(F32)
    bg = b[128:192].reshape(64, 1).astype(F32)
    hsb_o = (0.2 * b[192:256] + 0.5).reshape(64, 1).astype(F32)
    inv = (np.asarray(gamma, F32) /
           np.sqrt(np.asarray(moving_var, F32) + 1e-3))
    bns = inv.reshape(64, 1).astype(F32)
    bnb = (np.asarray(beta, F32) -
           np.asarray(moving_mean, F32) * inv).reshape(64, 1).astype(F32)

    in_maps = []
    for core in range(8):
        bidx, s = core // 4, core % 4
        r0 = 16 * s
        glo, ghi = max(0, r0 - 17), min(64, r0 + 33)
        i0 = glo - (r0 - 17)
        xpad = np.zeros((T, NR, WP, C), F32)
        xpad[:, i0:i0 + (ghi - glo), 1:65, :] = x[bidx, :, glo:ghi, :, :]
        xT = np.ascontiguousarray(
            xpad.transpose(0, 3, 1, 2).reshape(T, C, NCOL)).astype(BF16)
        m = np.zeros((NR, WP), F32)
        for i in range(NR):
            if 0 <= (r0 - 17 + i) < 64:
                m[i, 1:65] = 1.0
        mask = np.broadcast_to(m.reshape(1, NCOL), (64, NCOL)).copy()
        in_maps.append({
            "xT": xT, "w": wstack, "mask": mask, "hsb_if": hsb_if,
            "bg": bg, "hsb_o": hsb_o, "bns": bns, "bnb": bnb,
        })
    return in_maps


def kernel(x, Wx, Wh, b, gamma, beta, moving_mean, moving_var):
    global _PROG
    if _PROG is None:
        _PROG = _build()
    in_maps = _prep_inputs(x, Wx, Wh, b, gamma, beta, moving_mean, moving_var)
    res = run_bass_kernel_spmd(_PROG, in_maps, core_ids=list(range(8)))
    out = np.empty((2, T, 64, W, F), F32)
    for core in range(8):
        bidx, s = core // 4, core % 4
        oc = res.results[core]["out"].reshape(T, F, 16, WP)[:, :, :, 1:65]
        out[bidx, :, 16 * s:16 * s + 16] = oc.transpose(0, 2, 3, 1)
    return out
